# revision 1
# baseline (speedup 1.0000x reference)
"""ChannelAttention Trainium2 Bass kernel.

Full (unsharded) inputs -> full output. Data-parallel over batch B=8 across
the 8 NeuronCores (one batch element per core, SPMD program, no collectives).

Per-core math (N=4096 tokens, C=512 channels):
    qkv = x @ Wqkv + bqkv ; q,k,v = relu(split(qkv))
    scores = (q^T k) / sqrt(C)           # [C, C] contraction over tokens
    attn = softmax(scores, -1) * adj
    y = v @ attn ; out = y @ Wo + bo

Matmuls run in float32r (fp32 storage, ~1e-4 rel err, full PE rate).
"""

import sys

sys.path.insert(0, "/opt/trn_rl_repo")

from contextlib import ExitStack

import numpy as np

import concourse.bass as bass
import concourse.mybir as mybir
import concourse.tile as tile
from concourse import bacc
from concourse.bass import ds, ts
from concourse.bass_utils import run_bass_kernel_spmd
from concourse.masks import make_identity

# Problem shape (hardcoded per contract).
B, N, C = 8, 4096, 512
P = 128
CC = C // P            # channel chunks (4)
NT = N // P            # token tiles (32)
TPS = 4                # token tiles per slab
NS = NT // TPS         # slabs (8)
SLAB = TPS * P         # tokens per slab (512)

F32 = mybir.dt.float32
F32R = mybir.dt.float32r

_CACHE = {}


def build(reps: int = 1, mm_dt=None, tp_bufs=2, proj_bufs=2, qk_bufs=3,
          xin_bufs=3, xtp_bufs=2, y_bufs=4, pipe_p2=True):
    MMD = F32R if mm_dt is None else mm_dt
    nc = bacc.Bacc("TRN2", target_bir_lowering=False, debug=False, num_devices=8)

    x = nc.dram_tensor("x", [N, C], F32, kind="ExternalInput").ap()
    adj = nc.dram_tensor("adj", [C, C], F32, kind="ExternalInput").ap()
    wqkv = nc.dram_tensor("Wqkv", [C, 3 * C], F32, kind="ExternalInput").ap()
    bqkv = nc.dram_tensor("bqkv", [3 * C], F32, kind="ExternalInput").ap()
    wo = nc.dram_tensor("Wo", [C, C], F32, kind="ExternalInput").ap()
    bo = nc.dram_tensor("bo", [C], F32, kind="ExternalInput").ap()
    out = nc.dram_tensor("out", [N, C], F32, kind="ExternalOutput").ap()

    s = 1.0 / float(np.sqrt(C))

    with tile.TileContext(nc) as tc, ExitStack() as ctx:
        const = ctx.enter_context(tc.tile_pool(name="const", bufs=1))

        # ---- constants -------------------------------------------------
        with tc.tile_pool(name="stage", bufs=1) as stage:
            wqkv_f = stage.tile([P, CC, 3 * C], F32, tag="stage_wqkv")
            nc.sync.dma_start(wqkv_f[:], wqkv.rearrange("(o p) d -> p o d", p=P))
            wqkv_r = const.tile([P, CC, 3 * C], MMD)
            nc.vector.tensor_copy(wqkv_r[:], wqkv_f[:])

            wo_f = stage.tile([P, CC, C], F32, tag="stage_wo")
            nc.sync.dma_start(wo_f[:], wo.rearrange("(o p) d -> p o d", p=P))
            wo_r = const.tile([P, CC, C], MMD)
            nc.vector.tensor_copy(wo_r[:], wo_f[:])

            brow_f = stage.tile([1, 3 * C], F32, tag="stage_b")
            nc.sync.dma_start(brow_f[:], bqkv[None, :])
            brow_r = const.tile([1, 3 * C], MMD)
            nc.vector.tensor_copy(brow_r[:], brow_f[:])

            borow_f = stage.tile([1, C], F32, tag="stage_bo")
            nc.sync.dma_start(borow_f[:], bo[None, :])
            borow_r = const.tile([1, C], MMD)
            nc.vector.tensor_copy(borow_r[:], borow_f[:])

            ones_f = stage.tile([1, P], F32, tag="stage_ones")
            nc.gpsimd.memset(ones_f[:], 1.0)
            ones_r = const.tile([1, P], MMD)
            nc.vector.tensor_copy(ones_r[:], ones_f[:])

        # v-bias, per-partition layout [p, chunk]
        bv = const.tile([P, CC], F32)
        nc.sync.dma_start(bv[:], bqkv[2 * C :].rearrange("(o p) -> p o", p=P))

        ident = const.tile([P, P], F32)
        make_identity(nc, ident[:])

        adj_sb = const.tile([P, CC, C], F32)
        nc.sync.dma_start(adj_sb[:], adj.rearrange("(o p) d -> p o d", p=P))

        vt_sb = const.tile([P, CC, N], MMD)      # v^T, channel-major
        attn_sb = const.tile([P, CC, C], MMD)    # gated softmax rows

        # bo broadcast to [P, C] once (fold bias into pass-2 evacuation)
        bo_bc = const.tile([P, C], F32)
        with tc.tile_pool(name="bo_ps", bufs=1, space="PSUM") as bo_ps_pool:
            bo_ps = bo_ps_pool.tile([P, C], F32, name="bo_ps")
            nc.tensor.matmul(bo_ps[:], ones_r[:], borow_r[:], start=True, stop=True)
            nc.vector.tensor_copy(bo_bc[:], bo_ps[:])

        # ---- pass 1: qkv projection + channel scores -------------------
        scores_pool = ctx.enter_context(
            tc.tile_pool(name="scores", bufs=1, space="PSUM")
        )
        scores_ps = [
            scores_pool.tile([P, C], F32, tag=f"scores{o}", name=f"scores{o}")
            for o in range(CC)
        ]

        rep_ctx = tc.For_i(0, reps, 1) if reps > 1 else None
        if rep_ctx is not None:
            ctx.enter_context(rep_ctx)

        with (
            tc.tile_pool(name="tp_ps", bufs=tp_bufs, space="PSUM") as tp_ps,
            tc.tile_pool(name="proj_ps", bufs=proj_bufs, space="PSUM") as proj_ps,
            tc.tile_pool(name="xin", bufs=xin_bufs) as xin,
            tc.tile_pool(name="xtp", bufs=xtp_bufs) as xtp,
            tc.tile_pool(name="qk", bufs=qk_bufs) as qk,
        ):
            for sl in range(NS):
                xt_slab = xtp.tile([P, CC, SLAB], MMD, tag="xT")
                for tt in range(TPS):
                    t = sl * TPS + tt
                    x_t = xin.tile([P, C], F32, tag="x")
                    nc.sync.dma_start(x_t[:], x[ts(t, P), :])

                    # transpose 128x512 -> xT chunks via PE
                    pst = tp_ps.tile([P, C], F32, tag="tp")
                    for o in range(CC):
                        nc.tensor.transpose(pst[:, ts(o, P)], x_t[:, ts(o, P)], ident[:])
                    nc.vector.tensor_copy(
                        xt_slab[:, :, ts(tt, P)],
                        pst[:].rearrange("p (o n) -> p o n", o=CC),
                    )

                    # q = relu(x @ Wq + bq)   (token-major)
                    q_ps = proj_ps.tile([P, C], F32, tag="proj")
                    for o in range(CC):
                        nc.tensor.matmul(
                            q_ps[:],
                            xt_slab[:, o, ts(tt, P)],
                            wqkv_r[:, o, 0:C],
                            start=(o == 0),
                            stop=False,
                        )
                    nc.tensor.matmul(
                        q_ps[:], ones_r[:], brow_r[:, 0:C], start=False, stop=True
                    )
                    q_sb = qk.tile([P, C], MMD, tag="qk")
                    nc.scalar.activation(
                        q_sb[:], q_ps[:], mybir.ActivationFunctionType.Relu
                    )

                    # k = relu(x @ Wk + bk)
                    k_ps = proj_ps.tile([P, C], F32, tag="proj")
                    for o in range(CC):
                        nc.tensor.matmul(
                            k_ps[:],
                            xt_slab[:, o, ts(tt, P)],
                            wqkv_r[:, o, C : 2 * C],
                            start=(o == 0),
                            stop=False,
                        )
                    nc.tensor.matmul(
                        k_ps[:], ones_r[:], brow_r[:, C : 2 * C], start=False, stop=True
                    )
                    k_sb = qk.tile([P, C], MMD, tag="qk")
                    nc.vector.tensor_scalar_max(k_sb[:], k_ps[:], 0.0)

                    # scores[o] += q[:, o-chunk]^T @ k
                    for o in range(CC):
                        nc.tensor.matmul(
                            scores_ps[o][:],
                            q_sb[:, ts(o, P)],
                            k_sb[:],
                            start=(t == 0),
                            stop=(t == NT - 1),
                        )

                # vT[d, n] = relu(Wv^T x^T + bv)  (channel-major, kept in SBUF)
                for d in range(CC):
                    v_ps = proj_ps.tile([P, C], F32, tag="proj")
                    for o in range(CC):
                        nc.tensor.matmul(
                            v_ps[:, :SLAB],
                            wqkv_r[:, o, ds(2 * C + d * P, P)],
                            xt_slab[:, o, :],
                            start=(o == 0),
                            stop=(o == CC - 1),
                        )
                    nc.scalar.activation(
                        vt_sb[:, d, ts(sl, SLAB)],
                        v_ps[:, :SLAB],
                        mybir.ActivationFunctionType.Relu,
                        bias=bv[:, d : d + 1],
                    )

            # ---- softmax + adjacency gate ------------------------------
            with tc.tile_pool(name="smx", bufs=8) as smx:
                for o in range(CC):
                    smax = smx.tile([P, 1], F32, tag="smax")
                    nc.vector.reduce_max(
                        smax[:], scores_ps[o][:], axis=mybir.AxisListType.X
                    )
                    nbias = smx.tile([P, 1], F32, tag="nbias")
                    nc.vector.tensor_scalar_mul(nbias[:], smax[:], -s)
                    ssum = smx.tile([P, 1], F32, tag="ssum")
                    attn_e = smx.tile([P, C], F32, tag="attn_e")
                    nc.scalar.activation(
                        attn_e[:],
                        scores_ps[o][:],
                        mybir.ActivationFunctionType.Exp,
                        bias=nbias[:],
                        scale=s,
                        accum_out=ssum[:],
                    )
                    rsum = smx.tile([P, 1], F32, tag="rsum")
                    nc.vector.reciprocal(rsum[:], ssum[:])
                    attn_r = smx.tile([P, C], F32, tag="attn_r")
                    nc.vector.tensor_scalar_mul(attn_r[:], attn_e[:], rsum[:])
                    nc.vector.tensor_mul(
                        attn_sb[:, o, :], attn_r[:], adj_sb[:, o, :]
                    )

        # ---- pass 2: y = v @ attn ; out = y @ Wo + bo ------------------
        with (
            tc.tile_pool(name="y_ps", bufs=y_bufs, space="PSUM") as y_ps_pool,
            tc.tile_pool(name="yt", bufs=2) as ytp,
            tc.tile_pool(name="outp", bufs=3) as outp,
        ):
            def emit_yt(sl):
                yt_slab = ytp.tile([P, CC, SLAB], MMD, tag="yT", name=f"yt_{sl}")
                for d in range(CC):
                    y_ps = y_ps_pool.tile([P, C], F32, tag="y", name=f"y_{sl}_{d}")
                    for o in range(CC):
                        nc.tensor.matmul(
                            y_ps[:, :SLAB],
                            attn_sb[:, o, ts(d, P)],
                            vt_sb[:, o, ts(sl, SLAB)],
                            start=(o == 0),
                            stop=(o == CC - 1),
                        )
                    nc.scalar.copy(yt_slab[:, d, :], y_ps[:, :SLAB])
                return yt_slab

            def emit_out(sl, yt_slab):
                for tt in range(TPS):
                    t = sl * TPS + tt
                    o_ps = y_ps_pool.tile([P, C], F32, tag="y", name=f"o_{sl}_{tt}")
                    for d in range(CC):
                        nc.tensor.matmul(
                            o_ps[:],
                            yt_slab[:, d, ts(tt, P)],
                            wo_r[:, d, :],
                            start=(d == 0),
                            stop=(d == CC - 1),
                        )
                    out_sb = outp.tile([P, C], F32, tag="out", name=f"os_{sl}_{tt}")
                    nc.vector.tensor_tensor(
                        out_sb[:], o_ps[:], bo_bc[:], mybir.AluOpType.add
                    )
                    nc.sync.dma_start(out[ts(t, P), :], out_sb[:])

            if pipe_p2:
                prev = None
                for sl in range(NS):
                    yt_slab = emit_yt(sl)
                    if prev is not None:
                        emit_out(sl - 1, prev)
                    prev = yt_slab
                emit_out(NS - 1, prev)
            else:
                for sl in range(NS):
                    emit_out(sl, emit_yt(sl))

    nc.compile()
    return nc


def _get_nc(reps: int = 1, mm_dt=None, **kw):
    key = ("nc", reps, str(mm_dt), tuple(sorted(kw.items())))
    if key not in _CACHE:
        _CACHE[key] = build(reps, mm_dt, **kw)
    return _CACHE[key]


def _run(inputs, trace=False, reps: int = 1, mm_dt=None, **kw):
    nc = _get_nc(reps, mm_dt, **kw)
    x = np.ascontiguousarray(np.asarray(inputs["x"], dtype=np.float32))
    adj = np.ascontiguousarray(np.asarray(inputs["adj"], dtype=np.float32))
    wqkv = np.ascontiguousarray(np.asarray(inputs["Wqkv"], dtype=np.float32))
    bqkv = np.ascontiguousarray(np.asarray(inputs["bqkv"], dtype=np.float32))
    wo = np.ascontiguousarray(np.asarray(inputs["Wo"], dtype=np.float32))
    bo = np.ascontiguousarray(np.asarray(inputs["bo"], dtype=np.float32))

    in_maps = [
        {
            "x": x[b],
            "adj": adj[b],
            "Wqkv": wqkv,
            "bqkv": bqkv,
            "Wo": wo,
            "bo": bo,
        }
        for b in range(B)
    ]
    res = run_bass_kernel_spmd(
        nc, in_maps, core_ids=list(range(B)), trace=trace
    )
    outp = np.stack([res.results[b]["out"] for b in range(B)], axis=0)
    return outp.astype(np.float32), res


def kernel(**inputs) -> np.ndarray:
    out, _ = _run(inputs, trace=False)
    return out



# revision 2
# speedup vs baseline: 1.0723x; 1.0723x over previous
"""ChannelAttention Trainium2 Bass kernel.

Full (unsharded) inputs -> full output. Data-parallel over batch B=8 across
the 8 NeuronCores (one batch element per core, SPMD program, no collectives).

Per-core math (N=4096 tokens, C=512 channels):
    qkv = x @ Wqkv + bqkv ; q,k,v = relu(split(qkv))
    scores = (q^T k) / sqrt(C)           # [C, C] contraction over tokens
    attn = softmax(scores, -1) * adj
    y = v @ attn ; out = y @ Wo + bo

v2 design (PE-bound, ~768 N=512 matmuls):
  - all matmuls in bf16 (PSUM accumulate f32; rel err ~2e-3 vs 2e-2 gate)
  - x^T built with the DMA crossbar transpose (dma_start_transpose, bf16)
    instead of PE transposes: frees ~30us of PE and ~27us of DVE
  - q/k bias folded via DVE add into PSUM + Act relu (kills 64 bias matmuls)
  - scores matmuls run one token-tile behind q/k to hide Act/sem latency
  - v-projection deferred into pass 2 (overlaps the softmax stall) and
    interleaved two slabs ahead of y/out to hide relu/copy latency
"""

import sys

sys.path.insert(0, "/opt/trn_rl_repo")

from contextlib import ExitStack

import numpy as np

import concourse.bass as bass
import concourse.mybir as mybir
import concourse.tile as tile
from concourse import bacc
from concourse.bass import ds, ts
from concourse.bass_utils import run_bass_kernel_spmd

# Problem shape (hardcoded per contract).
B, N, C = 8, 4096, 512
P = 128
CC = C // P            # channel chunks (4)
NT = N // P            # token tiles (32)
NS = 8                 # pass-2 slabs
TPS = NT // NS         # token tiles per slab (4)
SLAB = TPS * P         # tokens per slab (512)

F32 = mybir.dt.float32
BF16 = mybir.dt.bfloat16
ADD = mybir.AluOpType.add
RELU = mybir.ActivationFunctionType.Relu

_CACHE = {}


def build(reps: int = 1):
    nc = bacc.Bacc("TRN2", target_bir_lowering=False, debug=False, num_devices=8)

    x = nc.dram_tensor("x", [N, C], F32, kind="ExternalInput").ap()
    adj = nc.dram_tensor("adj", [C, C], F32, kind="ExternalInput").ap()
    wqkv = nc.dram_tensor("Wqkv", [C, 3 * C], F32, kind="ExternalInput").ap()
    bqkv = nc.dram_tensor("bqkv", [3 * C], F32, kind="ExternalInput").ap()
    wo = nc.dram_tensor("Wo", [C, C], F32, kind="ExternalInput").ap()
    bo = nc.dram_tensor("bo", [C], F32, kind="ExternalInput").ap()
    out = nc.dram_tensor("out", [N, C], F32, kind="ExternalOutput").ap()

    s = 1.0 / float(np.sqrt(C))

    with tile.TileContext(nc) as tc, ExitStack() as ctx:
        const = ctx.enter_context(tc.tile_pool(name="const", bufs=1))

        # ---- constants -------------------------------------------------
        with tc.tile_pool(name="stage", bufs=1) as stage:
            wqkv_f = stage.tile([P, CC, 3 * C], F32, tag="stage_wqkv")
            nc.sync.dma_start(wqkv_f[:], wqkv.rearrange("(o p) d -> p o d", p=P))
            wqkv_r = const.tile([P, CC, 3 * C], BF16)
            nc.vector.tensor_copy(wqkv_r[:], wqkv_f[:])

            wo_f = stage.tile([P, CC, C], F32, tag="stage_wo")
            nc.sync.dma_start(wo_f[:], wo.rearrange("(o p) d -> p o d", p=P))
            wo_r = const.tile([P, CC, C], BF16)
            nc.vector.tensor_copy(wo_r[:], wo_f[:])

            brow_f = stage.tile([1, 2 * C], F32, tag="stage_b")
            nc.sync.dma_start(brow_f[:], bqkv[None, 0 : 2 * C])
            brow_r = stage.tile([1, 2 * C], BF16, tag="stage_br")
            nc.vector.tensor_copy(brow_r[:], brow_f[:])

            borow_f = stage.tile([1, C], F32, tag="stage_bo")
            nc.sync.dma_start(borow_f[:], bo[None, :])
            borow_r = stage.tile([1, C], BF16, tag="stage_bor")
            nc.vector.tensor_copy(borow_r[:], borow_f[:])

            ones_f = stage.tile([1, P], F32, tag="stage_ones")
            nc.gpsimd.memset(ones_f[:], 1.0)
            ones_r = stage.tile([1, P], BF16, tag="stage_onesr")
            nc.vector.tensor_copy(ones_r[:], ones_f[:])

            # broadcast biases to [P, *] once (read along free dim later)
            bias_qk = const.tile([P, 2 * C], F32)
            bo_bc = const.tile([P, C], F32)
            with tc.tile_pool(name="bc_ps", bufs=1, space="PSUM") as bc_pool:
                bq_ps = bc_pool.tile([P, C], F32, name="bq_ps", tag="b0")
                nc.tensor.matmul(bq_ps[:], ones_r[:], brow_r[:, 0:C], start=True, stop=True)
                nc.vector.tensor_copy(bias_qk[:, 0:C], bq_ps[:])
                bk_ps = bc_pool.tile([P, C], F32, name="bk_ps", tag="b1")
                nc.tensor.matmul(bk_ps[:], ones_r[:], brow_r[:, C : 2 * C], start=True, stop=True)
                nc.vector.tensor_copy(bias_qk[:, C : 2 * C], bk_ps[:])
                bo_ps = bc_pool.tile([P, C], F32, name="bo_ps", tag="b2")
                nc.tensor.matmul(bo_ps[:], ones_r[:], borow_r[:], start=True, stop=True)
                nc.vector.tensor_copy(bo_bc[:], bo_ps[:])

        # v-bias, per-partition layout [p, chunk]
        bv = const.tile([P, CC], F32)
        nc.sync.dma_start(bv[:], bqkv[2 * C :].rearrange("(o p) -> p o", p=P))

        adj_sb = const.tile([P, CC, C], F32)
        nc.sync.dma_start(adj_sb[:], adj.rearrange("(o p) d -> p o d", p=P))

        xt_all = const.tile([P, CC, N], BF16)    # x^T, channel-major (32KB/part)
        attn_sb = const.tile([P, CC, C], BF16)   # gated softmax rows

        # ---- per-iteration body ---------------------------------------
        scores_pool = ctx.enter_context(
            tc.tile_pool(name="scores", bufs=1, space="PSUM")
        )
        scores_ps = [
            scores_pool.tile([P, C], F32, tag=f"scores{o}", name=f"scores{o}")
            for o in range(CC)
        ]

        rep_ctx = tc.For_i(0, reps, 1) if reps > 1 else None
        if rep_ctx is not None:
            ctx.enter_context(rep_ctx)

        # ---- pass 1: x^T staging, q/k projection, channel scores ------
        with (
            tc.tile_pool(name="proj_ps", bufs=3, space="PSUM") as proj_ps,
            tc.tile_pool(name="xin", bufs=4) as xin,
            tc.tile_pool(name="xb", bufs=3) as xbp,
            tc.tile_pool(name="qk", bufs=4) as qk,
        ):
            def load_x(t):
                x_t = xin.tile([P, C], F32, tag="x", name=f"x_{t}")
                nc.sync.dma_start(x_t[:], x[ts(t, P), :])
                return x_t

            def prep_xt(t, x_t):
                x_b = xbp.tile([P, C], BF16, tag="xb", name=f"xb_{t}")
                nc.vector.tensor_copy(x_b[:], x_t[:])
                nc.sync.dma_start_transpose(xt_all[:, :, ts(t, P)], x_b[:])

            def proj_qk(t):
                q_ps = proj_ps.tile([P, C], F32, tag="proj", name=f"q_{t}")
                for o in range(CC):
                    nc.tensor.matmul(
                        q_ps[:], xt_all[:, o, ts(t, P)], wqkv_r[:, o, 0:C],
                        start=(o == 0), stop=(o == CC - 1),
                    )
                k_ps = proj_ps.tile([P, C], F32, tag="proj", name=f"k_{t}")
                for o in range(CC):
                    nc.tensor.matmul(
                        k_ps[:], xt_all[:, o, ts(t, P)], wqkv_r[:, o, C : 2 * C],
                        start=(o == 0), stop=(o == CC - 1),
                    )
                nc.vector.tensor_tensor(q_ps[:], q_ps[:], bias_qk[:, 0:C], ADD)
                q_sb = qk.tile([P, C], BF16, tag="qk", name=f"qs_{t}")
                nc.scalar.activation(q_sb[:], q_ps[:], RELU)
                nc.vector.tensor_tensor(k_ps[:], k_ps[:], bias_qk[:, C : 2 * C], ADD)
                k_sb = qk.tile([P, C], BF16, tag="qk", name=f"ks_{t}")
                nc.scalar.activation(k_sb[:], k_ps[:], RELU)
                return q_sb, k_sb

            def scores_mm(t, q_sb, k_sb):
                for o in range(CC):
                    nc.tensor.matmul(
                        scores_ps[o][:], q_sb[:, ts(o, P)], k_sb[:],
                        start=(t == 0), stop=(t == NT - 1),
                    )

            x_tiles = {t: load_x(t) for t in range(3)}
            prep_xt(0, x_tiles[0])
            prep_xt(1, x_tiles[1])
            prev_qk = None
            for t in range(NT):
                if t + 3 < NT:
                    x_tiles[t + 3] = load_x(t + 3)
                if t + 2 < NT:
                    prep_xt(t + 2, x_tiles.pop(t + 2))
                q_sb, k_sb = proj_qk(t)
                if prev_qk is not None:
                    scores_mm(t - 1, *prev_qk)
                prev_qk = (q_sb, k_sb)
            scores_mm(NT - 1, *prev_qk)

        # ---- softmax + adjacency gate (overlaps pass-2 v matmuls) ------
        with (
            tc.tile_pool(name="smx", bufs=8) as smx,
            tc.tile_pool(name="v_ps", bufs=2, space="PSUM") as v_ps_pool,
            tc.tile_pool(name="yo_ps", bufs=2, space="PSUM") as yo_ps_pool,
            tc.tile_pool(name="vt", bufs=3) as vtp,
            tc.tile_pool(name="yt", bufs=2) as ytp,
            tc.tile_pool(name="outp", bufs=3) as outp,
        ):
            def softmax_chunk(o):
                smax = smx.tile([P, 1], F32, tag="smax")
                nc.vector.reduce_max(
                    smax[:], scores_ps[o][:], axis=mybir.AxisListType.X
                )
                nbias = smx.tile([P, 1], F32, tag="nbias")
                nc.vector.tensor_scalar_mul(nbias[:], smax[:], -s)
                ssum = smx.tile([P, 1], F32, tag="ssum")
                attn_e = smx.tile([P, C], F32, tag="attn_e")
                nc.scalar.activation(
                    attn_e[:], scores_ps[o][:],
                    mybir.ActivationFunctionType.Exp,
                    bias=nbias[:], scale=s, accum_out=ssum[:],
                )
                rsum = smx.tile([P, 1], F32, tag="rsum")
                nc.vector.reciprocal(rsum[:], ssum[:])
                attn_r = smx.tile([P, C], F32, tag="attn_r")
                nc.vector.tensor_scalar_mul(attn_r[:], attn_e[:], rsum[:])
                nc.vector.tensor_mul(attn_sb[:, o, :], attn_r[:], adj_sb[:, o, :])

            # ---- pass 2: v^T projection, y = v @ attn, out = y @ Wo + bo
            def emit_v(sl):
                vt_slab = vtp.tile([P, CC, SLAB], BF16, tag="vT", name=f"vt_{sl}")
                for d in range(CC):
                    v_ps = v_ps_pool.tile([P, SLAB], F32, tag="v", name=f"v_{sl}_{d}")
                    for o in range(CC):
                        nc.tensor.matmul(
                            v_ps[:],
                            wqkv_r[:, o, ds(2 * C + d * P, P)],
                            xt_all[:, o, ts(sl, SLAB)],
                            start=(o == 0), stop=(o == CC - 1),
                        )
                    nc.scalar.activation(
                        vt_slab[:, d, :], v_ps[:], RELU, bias=bv[:, d : d + 1]
                    )
                return vt_slab

            def emit_y(sl, vt_slab):
                yt_slab = ytp.tile([P, CC, SLAB], BF16, tag="yT", name=f"yt_{sl}")
                for d in range(CC):
                    y_ps = yo_ps_pool.tile([P, SLAB], F32, tag="yo", name=f"y_{sl}_{d}")
                    for o in range(CC):
                        nc.tensor.matmul(
                            y_ps[:],
                            attn_sb[:, o, ts(d, P)],
                            vt_slab[:, o, :],
                            start=(o == 0), stop=(o == CC - 1),
                        )
                    nc.scalar.copy(yt_slab[:, d, :], y_ps[:])
                return yt_slab

            def emit_out(sl, yt_slab):
                for tt in range(TPS):
                    t = sl * TPS + tt
                    o_ps = yo_ps_pool.tile([P, C], F32, tag="yo", name=f"o_{sl}_{tt}")
                    for d in range(CC):
                        nc.tensor.matmul(
                            o_ps[:],
                            yt_slab[:, d, ts(tt, P)],
                            wo_r[:, d, :],
                            start=(d == 0), stop=(d == CC - 1),
                        )
                    out_sb = outp.tile([P, C], F32, tag="out", name=f"os_{sl}_{tt}")
                    nc.vector.tensor_tensor(out_sb[:], o_ps[:], bo_bc[:], ADD)
                    nc.sync.dma_start(out[ts(t, P), :], out_sb[:])

            vt = {0: emit_v(0)}
            for o in range(CC):
                softmax_chunk(o)
            vt[1] = emit_v(1)
            yt = {}
            for sl in range(NS):
                yt[sl] = emit_y(sl, vt.pop(sl))
                if sl + 2 < NS:
                    vt[sl + 2] = emit_v(sl + 2)
                emit_out(sl, yt.pop(sl))

    nc.compile()
    return nc


def _get_nc(reps: int = 1, **kw):
    key = ("nc", reps, tuple(sorted(kw.items())))
    if key not in _CACHE:
        _CACHE[key] = build(reps, **kw)
    return _CACHE[key]


def _run(inputs, trace=False, reps: int = 1, **kw):
    nc = _get_nc(reps, **kw)
    x = np.ascontiguousarray(np.asarray(inputs["x"], dtype=np.float32))
    adj = np.ascontiguousarray(np.asarray(inputs["adj"], dtype=np.float32))
    wqkv = np.ascontiguousarray(np.asarray(inputs["Wqkv"], dtype=np.float32))
    bqkv = np.ascontiguousarray(np.asarray(inputs["bqkv"], dtype=np.float32))
    wo = np.ascontiguousarray(np.asarray(inputs["Wo"], dtype=np.float32))
    bo = np.ascontiguousarray(np.asarray(inputs["bo"], dtype=np.float32))

    in_maps = [
        {
            "x": x[b],
            "adj": adj[b],
            "Wqkv": wqkv,
            "bqkv": bqkv,
            "Wo": wo,
            "bo": bo,
        }
        for b in range(B)
    ]
    res = run_bass_kernel_spmd(
        nc, in_maps, core_ids=list(range(B)), trace=trace
    )
    outp = np.stack([res.results[b]["out"] for b in range(B)], axis=0)
    return outp.astype(np.float32), res


def kernel(**inputs) -> np.ndarray:
    out, _ = _run(inputs, trace=False)
    return out
